# revision 1
# baseline (speedup 1.0000x reference)
"""MoE fusion kernel for Trainium2 (8 NeuronCores, data-parallel over tokens).

Strategy
--------
Data-parallel: each of the 8 cores gets 1024 tokens and runs the full model
(gate + all 12 experts) on its shard.  No collectives needed.

Everything on-device runs in a feature-major ("transposed") layout so that
both expert matmuls use weights as the stationary operand and activations as
the moving operand -- no transposes of the big intermediates are ever needed:

    x.T [1536, T] -> h.T = gelu(W1.T x.T) [3072, T] -> o.T = W2.T h.T [768, T]

The gate must be computed in true fp32 (4-pass PE matmul): the smallest gap
between the 2nd and 3rd gate logit over the 8192 tokens is ~3.5e-5, so any
low-precision gate (bf16 / fp22) would flip top-k selections.  Expert MLPs
run in bf16 (their error does not affect routing, only adds ~1e-3 output
noise, well within tolerance).

Top-2 selection uses the DVE max8 instruction; the normalized top-2 softmax
weights collapse to sigmoid(l1-l2), computed as 0.5*(1+tanh(0.5*(l1-l2))) so
the whole kernel needs only the one 'gelu_and_others' ACT table set (gelu +
tanh).  The final sigmoid is likewise computed via tanh and folded as
fused = 0.5 + 0.5 * sum_e w_e * tanh(0.5*(o_e + b2_e)), using sum_e w_e = 1.
"""

import numpy as np

try:
    import concourse  # noqa: F401
except ImportError:  # pragma: no cover
    import sys

    sys.path.insert(0, "/opt/trn_rl_repo")

import concourse.bass as bass  # noqa: F401
import concourse.mybir as mybir
import concourse.tile as tile
from concourse import bacc
from concourse.bass_utils import run_bass_kernel_spmd

# Problem shapes (hardcoded per contest rules).
N, D, E, H, NE = 8192, 1536, 768, 3072, 12
NCORES = 8
T = N // NCORES  # 1024 tokens per core
P = 128
KO1 = D // P  # 12   k-tiles of the first expert matmul
FO1 = H // P  # 24   feature-tiles of h
KO2 = H // P  # 24   k-tiles of the second expert matmul
FO2 = E // P  # 6    feature-tiles of the output
GFO = E // P  # 6    feature-tiles of the gate hidden
TT = T // 512  # 2   512-token moving-operand chunks

F32 = mybir.dt.float32
BF16 = mybir.dt.bfloat16
AF = mybir.ActivationFunctionType
OP = mybir.AluOpType

USE_GPSIMD_BCAST = False  # partition_broadcast vs PE ones-matmul fallback
GELU = AF.Gelu  # test.py sim-mode substitutes Tanh (CoreSim lacks Gelu)


def _emit(tc, aps):
    nc = tc.nc
    (xT, xTb, gw1, gb1, gw2, gb2r, w1e, b1e, w2e, b2e, iden, out) = aps

    import contextlib

    with contextlib.ExitStack() as ctx:
        # ---------------- persistent tensors ----------------
        pers = ctx.enter_context(tc.tile_pool(name="pers", bufs=1))
        xTb_s = pers.tile([P, KO1, T], BF16)
        nc.sync.dma_start(xTb_s[:], xTb)
        b1e_s = pers.tile([P, NE, FO1], F32)
        nc.sync.dma_start(b1e_s[:], b1e)
        b2e_s = pers.tile([P, NE, FO2], F32)
        nc.sync.dma_start(b2e_s[:], b2e)
        acc = pers.tile([P, FO2, T], F32)
        wT = pers.tile([NE, T], F32)  # per-expert combine weights, feature-major
        ones_sb = None
        if not USE_GPSIMD_BCAST:
            ones_sb = pers.tile([1, P], F32)
            nc.vector.memset(ones_sb[:], 1.0)

        # ---------------- gate (scoped pools; space reused later) -------------
        with (
            tc.tile_pool(name="gate_sb", bufs=1) as gsb,
            tc.tile_pool(name="gate_tmp", bufs=2) as gtmp,
            tc.tile_pool(name="gate_ps", bufs=2, space="PSUM") as gps,
            tc.tile_pool(name="gate_ps_small", bufs=2, space="PSUM") as gpss,
        ):
            xT_s = gsb.tile([P, KO1, T], F32)
            nc.sync.dma_start(xT_s[:], xT)
            gw1_s = gsb.tile([P, KO1, E], F32)
            nc.sync.dma_start(gw1_s[:], gw1)
            gb1_s = gsb.tile([P, GFO], F32)
            nc.sync.dma_start(gb1_s[:], gb1)
            gw2_s = gsb.tile([P, GFO, NE], F32)
            nc.sync.dma_start(gw2_s[:], gw2)
            gb2r_s = gsb.tile([P, NE], F32)
            nc.sync.dma_start(gb2r_s[:], gb2r)
            iden_s = gsb.tile([P, P], F32)
            nc.sync.dma_start(iden_s[:], iden)
            ghT = gsb.tile([P, GFO, T], F32)

            # gh.T = gelu(gw1.T @ x.T + gb1)   (true fp32 matmuls)
            for fo in range(GFO):
                pg = gps.tile([P, T], F32, tag="gps")
                for t2 in range(TT):
                    for ko in range(KO1):
                        nc.tensor.matmul(
                            pg[:, t2 * 512 : (t2 + 1) * 512],
                            lhsT=gw1_s[:, ko, fo * P : (fo + 1) * P],
                            rhs=xT_s[:, ko, t2 * 512 : (t2 + 1) * 512],
                            start=(ko == 0),
                            stop=(ko == KO1 - 1),
                        )
                nc.scalar.activation(
                    ghT[:, fo, :], pg[:], GELU, bias=gb1_s[:, fo : fo + 1]
                )

            # logits (token-major) + top-2 -> combine weights, transposed to wT
            for tt in range(T // P):
                pl = gpss.tile([P, NE], F32, tag="gpl")
                for fo in range(GFO):
                    nc.tensor.matmul(
                        pl[:],
                        lhsT=ghT[:, fo, tt * P : (tt + 1) * P],
                        rhs=gw2_s[:, fo, :],
                        start=(fo == 0),
                        stop=(fo == GFO - 1),
                    )
                lt = gtmp.tile([P, NE], F32, tag="lt")
                nc.vector.tensor_tensor(lt[:], pl[:], gb2r_s[:], OP.add)
                m8 = gtmp.tile([P, 8], F32, tag="m8")
                nc.vector.max(m8[:], lt[:])
                dlt = gtmp.tile([P, 1], F32, tag="dlt")
                nc.vector.tensor_tensor(dlt[:], m8[:, 0:1], m8[:, 1:2], OP.subtract)
                w1v = gtmp.tile([P, 1], F32, tag="w1v")
                # w1 = sigmoid(l1-l2) = 0.5 + 0.5*tanh(0.5*(l1-l2))
                nc.scalar.activation(w1v[:], dlt[:], AF.Tanh, scale=0.5)
                nc.vector.tensor_scalar(w1v[:], w1v[:], 0.5, 0.5, OP.mult, OP.add)
                w2v = gtmp.tile([P, 1], F32, tag="w2v")
                nc.vector.tensor_scalar(w2v[:], w1v[:], -1.0, 1.0, OP.mult, OP.add)
                eq1 = gtmp.tile([P, NE], F32, tag="eq1")
                nc.vector.tensor_scalar(eq1[:], lt[:], m8[:, 0:1], None, OP.is_equal)
                nc.vector.tensor_scalar(eq1[:], eq1[:], w1v[:], None, OP.mult)
                eq2 = gtmp.tile([P, NE], F32, tag="eq2")
                nc.vector.tensor_scalar(eq2[:], lt[:], m8[:, 1:2], None, OP.is_equal)
                nc.vector.tensor_scalar(eq2[:], eq2[:], w2v[:], None, OP.mult)
                nc.vector.tensor_tensor(eq1[:], eq1[:], eq2[:], OP.add)
                ptw = gpss.tile([NE, P], F32, tag="gpt")
                nc.tensor.transpose(ptw[:], eq1[:], iden_s[:])
                nc.vector.tensor_copy(wT[:, tt * P : (tt + 1) * P], ptw[:])

        # ---------------- experts ----------------
        w1pool = ctx.enter_context(tc.tile_pool(name="w1p", bufs=3))
        w2pool = ctx.enter_context(tc.tile_pool(name="w2p", bufs=3))
        hpool = ctx.enter_context(tc.tile_pool(name="hp", bufs=FO1 + 4))
        wbpool = ctx.enter_context(tc.tile_pool(name="wbp", bufs=2))
        spool = ctx.enter_context(tc.tile_pool(name="sp", bufs=2))
        tpool = ctx.enter_context(tc.tile_pool(name="tp", bufs=2))
        psA = ctx.enter_context(tc.tile_pool(name="psA", bufs=2, space="PSUM"))
        psB = ctx.enter_context(tc.tile_pool(name="psB", bufs=2, space="PSUM"))

        for e in range(NE):
            wb = wbpool.tile([P, T], F32, tag="wb")
            # move this expert's weight row to partition 0, then replicate
            # across all 128 partitions
            wrow = wbpool.tile([1, T], F32, tag="wrow")
            nc.sync.dma_start(wrow[:], wT[e : e + 1, :])
            if USE_GPSIMD_BCAST:
                nc.gpsimd.partition_broadcast(wb[:], wrow[:])
            else:
                # rank-1 ones-outer-product broadcast on the PE
                pwb = psA.tile([P, T], F32, tag="psA")
                for t2 in range(TT):
                    nc.tensor.matmul(
                        pwb[:, t2 * 512 : (t2 + 1) * 512],
                        lhsT=ones_sb[:],
                        rhs=wrow[:, t2 * 512 : (t2 + 1) * 512],
                        start=True,
                        stop=True,
                    )
                nc.vector.tensor_copy(wb[:], pwb[:])

            hts = []
            for fo in range(FO1):
                w1t = w1pool.tile([P, KO1, P], BF16, tag="w1t")
                nc.sync.dma_start(w1t[:], w1e[e, fo])
                pa = psA.tile([P, T], F32, tag="psA")
                for ko in range(KO1):
                    for t2 in range(TT):
                        nc.tensor.matmul(
                            pa[:, t2 * 512 : (t2 + 1) * 512],
                            lhsT=w1t[:, ko, :],
                            rhs=xTb_s[:, ko, t2 * 512 : (t2 + 1) * 512],
                            start=(ko == 0),
                            stop=(ko == KO1 - 1),
                        )
                ht = hpool.tile([P, T], BF16, tag="ht")
                nc.scalar.activation(
                    ht[:], pa[:], GELU, bias=b1e_s[:, e, fo : fo + 1]
                )
                hts.append(ht)

            for fo2 in range(FO2):
                w2t = w2pool.tile([P, KO2, P], BF16, tag="w2t")
                nc.sync.dma_start(w2t[:], w2e[e, fo2])
                pb = psB.tile([P, T], F32, tag="psB")
                for ko in range(KO2):
                    for t2 in range(TT):
                        nc.tensor.matmul(
                            pb[:, t2 * 512 : (t2 + 1) * 512],
                            lhsT=w2t[:, ko, :],
                            rhs=hts[ko][:, t2 * 512 : (t2 + 1) * 512],
                            start=(ko == 0),
                            stop=(ko == KO2 - 1),
                        )
                st = spool.tile([P, T], F32, tag="st")
                # tanh(0.5*o + 0.5*b2)  (b2e input is pre-halved on host)
                nc.scalar.activation(
                    st[:], pb[:], AF.Tanh, bias=b2e_s[:, e, fo2 : fo2 + 1], scale=0.5
                )
                if e == 0:
                    nc.vector.tensor_tensor(acc[:, fo2, :], st[:], wb[:], OP.mult)
                else:
                    tmp = tpool.tile([P, T], F32, tag="tmp")
                    nc.vector.tensor_tensor(tmp[:], st[:], wb[:], OP.mult)
                    nc.vector.tensor_tensor(
                        acc[:, fo2, :], acc[:, fo2, :], tmp[:], OP.add
                    )

        # fused = 0.5 + 0.5 * acc  (sum of weights is 1)
        for fo2 in range(FO2):
            fin = tpool.tile([P, T], F32, tag="fin")
            nc.vector.tensor_scalar(fin[:], acc[:, fo2, :], 0.5, 0.5, OP.mult, OP.add)
            nc.sync.dma_start(out[:, fo2, :], fin[:])


def build_nc():
    nc = bacc.Bacc(
        "TRN2", target_bir_lowering=False, debug=False, num_devices=NCORES
    )
    aps = (
        nc.dram_tensor("xT", [P, KO1, T], F32, kind="ExternalInput").ap(),
        nc.dram_tensor("xTb", [P, KO1, T], BF16, kind="ExternalInput").ap(),
        nc.dram_tensor("gw1", [P, KO1, E], F32, kind="ExternalInput").ap(),
        nc.dram_tensor("gb1", [P, GFO], F32, kind="ExternalInput").ap(),
        nc.dram_tensor("gw2", [P, GFO, NE], F32, kind="ExternalInput").ap(),
        nc.dram_tensor("gb2r", [P, NE], F32, kind="ExternalInput").ap(),
        nc.dram_tensor("w1e", [NE, FO1, P, KO1, P], BF16, kind="ExternalInput").ap(),
        nc.dram_tensor("b1e", [P, NE, FO1], F32, kind="ExternalInput").ap(),
        nc.dram_tensor("w2e", [NE, FO2, P, KO2, P], BF16, kind="ExternalInput").ap(),
        nc.dram_tensor("b2e", [P, NE, FO2], F32, kind="ExternalInput").ap(),
        nc.dram_tensor("iden", [P, P], F32, kind="ExternalInput").ap(),
        nc.dram_tensor("accT", [P, FO2, T], F32, kind="ExternalOutput").ap(),
    )
    with tile.TileContext(nc) as tc:
        _emit(tc, aps)
    nc.compile()
    return nc


def prep_inputs(inputs):
    """Host-side sharding / relayout.  Returns (shared_map, per_core_xT, per_core_xTb)."""
    bf16 = mybir.dt.np(BF16)
    combined = np.asarray(inputs["combined"], np.float32)
    gate_w1 = np.asarray(inputs["gate_w1"], np.float32)
    gate_b1 = np.asarray(inputs["gate_b1"], np.float32)
    gate_w2 = np.asarray(inputs["gate_w2"], np.float32)
    gate_b2 = np.asarray(inputs["gate_b2"], np.float32)
    ew1 = np.asarray(inputs["ew1"], np.float32)
    eb1 = np.asarray(inputs["eb1"], np.float32)
    ew2 = np.asarray(inputs["ew2"], np.float32)
    eb2 = np.asarray(inputs["eb2"], np.float32)

    shared = {
        "gw1": np.ascontiguousarray(
            gate_w1.reshape(KO1, P, E).transpose(1, 0, 2)
        ),
        "gb1": np.ascontiguousarray(gate_b1.reshape(GFO, P).T),
        "gw2": np.ascontiguousarray(gate_w2.reshape(GFO, P, NE).transpose(1, 0, 2)),
        "gb2r": np.ascontiguousarray(np.broadcast_to(gate_b2, (P, NE))),
        "w1e": np.ascontiguousarray(
            ew1.reshape(NE, KO1, P, FO1, P).transpose(0, 3, 2, 1, 4)
        ).astype(bf16),
        "b1e": np.ascontiguousarray(eb1.reshape(NE, FO1, P).transpose(2, 0, 1)),
        "w2e": np.ascontiguousarray(
            ew2.reshape(NE, KO2, P, FO2, P).transpose(0, 3, 2, 1, 4)
        ).astype(bf16),
        "b2e": np.ascontiguousarray(
            (0.5 * eb2).reshape(NE, FO2, P).transpose(2, 0, 1)
        ),
        "iden": np.eye(P, dtype=np.float32),
    }
    xTs, xTbs = [], []
    for c in range(NCORES):
        xt = np.ascontiguousarray(
            combined[c * T : (c + 1) * T].T.reshape(KO1, P, T).transpose(1, 0, 2)
        )
        xTs.append(xt)
        xTbs.append(np.ascontiguousarray(xt.astype(bf16)))
    return shared, xTs, xTbs


_NC_CACHE = {}


def kernel_dense(**inputs):
    if "nc" not in _NC_CACHE:
        _NC_CACHE["nc"] = build_nc()
    nc = _NC_CACHE["nc"]

    shared, xTs, xTbs = prep_inputs(inputs)
    in_maps = [
        {**shared, "xT": xTs[c], "xTb": xTbs[c]} for c in range(NCORES)
    ]
    res = run_bass_kernel_spmd(nc, in_maps, core_ids=list(range(NCORES)))
    outs = res.results

    fused = np.empty((N, E), np.float32)
    for c in range(NCORES):
        accT = outs[c]["accT"]  # [P, FO2, T]
        fused[c * T : (c + 1) * T] = accT.transpose(2, 1, 0).reshape(T, E)
    return fused


# ======================================================================
# Sparse (true MoE routing) two-phase path.
#
# Phase 1 computes the gate logits on device (fp32).  The host does
# softmax / top-2 / routing (trivial [8192,12] work -- this is the shard
# assignment for phase 2, all FLOPs stay on device).  Phase 2 runs only
# the selected (token, expert) pairs: each expert's tokens are split
# across 2 cores (3 expert-slots per core, uniform slot capacity S so
# the SPMD program is core-uniform; per-core weight *inputs* carry each
# core's 3 experts).  Outputs come back pre-weighted by the combine
# weight; the host scatter-adds slot outputs into the [N, E] result.
# This executes ~TOPK/NE = 1/6 of the dense expert FLOPs.
# ======================================================================

EXP = 5  # expert slots per core; 8*5 = 40 slots, assigned to experts by load
S_MAX = 1024  # beyond this the phase-2 working set won't fit SBUF -> dense


def _chunks(total, step=512):
    return [(a, min(a + step, total)) for a in range(0, total, step)]


def build_nc_gate():
    nc = bacc.Bacc("TRN2", target_bir_lowering=False, debug=False, num_devices=NCORES)
    xT = nc.dram_tensor("xT", [P, KO1, T], F32, kind="ExternalInput").ap()
    gw1 = nc.dram_tensor("gw1", [P, KO1, E], F32, kind="ExternalInput").ap()
    gb1 = nc.dram_tensor("gb1", [P, GFO], F32, kind="ExternalInput").ap()
    gw2 = nc.dram_tensor("gw2", [P, GFO, NE], F32, kind="ExternalInput").ap()
    gb2r = nc.dram_tensor("gb2r", [P, NE], F32, kind="ExternalInput").ap()
    lg = nc.dram_tensor("lg", [P, T // P, NE], F32, kind="ExternalOutput").ap()

    with tile.TileContext(nc) as tc:
        with (
            tc.tile_pool(name="sb", bufs=1) as sb,
            tc.tile_pool(name="tmp", bufs=3) as tmp,
            tc.tile_pool(name="ps", bufs=2, space="PSUM") as ps,
            tc.tile_pool(name="pss", bufs=2, space="PSUM") as pss,
        ):
            xT_s = sb.tile([P, KO1, T], F32)
            gw1_s = sb.tile([P, KO1, E], F32)
            for ko in range(KO1):
                nc.sync.dma_start(xT_s[:, ko, :], xT[:, ko, :])
                nc.sync.dma_start(gw1_s[:, ko, :], gw1[:, ko, :])
            gb1_s = sb.tile([P, GFO], F32)
            nc.sync.dma_start(gb1_s[:], gb1)
            gw2_s = sb.tile([P, GFO, NE], F32)
            nc.sync.dma_start(gw2_s[:], gw2)
            gb2r_s = sb.tile([P, NE], F32)
            nc.sync.dma_start(gb2r_s[:], gb2r)
            ghT = sb.tile([P, GFO, T], F32)

            for fo in range(GFO):
                pg = ps.tile([P, T], F32, tag="pg")
                for a, b in _chunks(T):
                    for ko in range(KO1):
                        nc.tensor.matmul(
                            pg[:, a:b],
                            lhsT=gw1_s[:, ko, fo * P : (fo + 1) * P],
                            rhs=xT_s[:, ko, a:b],
                            start=(ko == 0),
                            stop=(ko == KO1 - 1),
                        )
                nc.scalar.activation(
                    ghT[:, fo, :], pg[:], GELU, bias=gb1_s[:, fo : fo + 1]
                )
            for tt in range(T // P):
                pl = pss.tile([P, NE], F32, tag="pl")
                for fo in range(GFO):
                    nc.tensor.matmul(
                        pl[:],
                        lhsT=ghT[:, fo, tt * P : (tt + 1) * P],
                        rhs=gw2_s[:, fo, :],
                        start=(fo == 0),
                        stop=(fo == GFO - 1),
                    )
                lt = tmp.tile([P, NE], F32, tag="lt")
                nc.vector.tensor_tensor(lt[:], pl[:], gb2r_s[:], OP.add)
                nc.sync.dma_start(lg[:, tt, :], lt[:])
    nc.compile()
    return nc


def build_nc_exp(S):
    Tc = EXP * S
    nc = bacc.Bacc("TRN2", target_bir_lowering=False, debug=False, num_devices=NCORES)
    xTe = nc.dram_tensor("xTe", [P, KO1, Tc], BF16, kind="ExternalInput").ap()
    wrow = nc.dram_tensor("wrow", [1, Tc], F32, kind="ExternalInput").ap()
    w1s = nc.dram_tensor("w1s", [EXP, FO1, P, KO1, P], BF16, kind="ExternalInput").ap()
    b1s = nc.dram_tensor("b1s", [P, EXP, FO1], F32, kind="ExternalInput").ap()
    w2s = nc.dram_tensor("w2s", [EXP, FO2, P, KO2, P], BF16, kind="ExternalInput").ap()
    b2s = nc.dram_tensor("b2s", [P, EXP, FO2], F32, kind="ExternalInput").ap()
    oT = nc.dram_tensor("oT", [P, FO2, Tc], F32, kind="ExternalOutput").ap()

    import contextlib

    with tile.TileContext(nc) as tc, contextlib.ExitStack() as ctx:
        pers = ctx.enter_context(tc.tile_pool(name="pers", bufs=1))
        xTe_s = pers.tile([P, KO1 * Tc], BF16)
        for ko in range(KO1):
            nc.sync.dma_start(xTe_s[:, ko * Tc : (ko + 1) * Tc], xTe[:, ko, :])
        b1s_s = pers.tile([P, EXP, FO1], F32)
        nc.sync.dma_start(b1s_s[:], b1s)
        b2s_s = pers.tile([P, EXP, FO2], F32)
        nc.sync.dma_start(b2s_s[:], b2s)
        wb2 = pers.tile([P, Tc], F32)

        with (
            tc.tile_pool(name="bc", bufs=1) as bc,
            tc.tile_pool(name="bcp", bufs=2, space="PSUM") as bcp,
        ):
            ones_sb = bc.tile([1, P], F32)
            nc.vector.memset(ones_sb[:], 1.0)
            wrow_s = bc.tile([1, Tc], F32)
            nc.sync.dma_start(wrow_s[:], wrow)
            for a, b in _chunks(Tc):
                pw = bcp.tile([P, 512], F32, tag="pw")
                nc.tensor.matmul(
                    pw[:, : b - a],
                    lhsT=ones_sb[:],
                    rhs=wrow_s[:, a:b],
                    start=True,
                    stop=True,
                )
                nc.scalar.mul(wb2[:, a:b], pw[:, : b - a], 0.5)  # wb2 = w/2

        w1pool = ctx.enter_context(tc.tile_pool(name="w1p", bufs=4))
        w2pool = ctx.enter_context(tc.tile_pool(name="w2p", bufs=2))
        hpool = ctx.enter_context(tc.tile_pool(name="hp", bufs=1))
        spool = ctx.enter_context(tc.tile_pool(name="sp", bufs=2))
        tpool = ctx.enter_context(tc.tile_pool(name="tp", bufs=2))
        psA = ctx.enter_context(tc.tile_pool(name="psA", bufs=2, space="PSUM"))
        psB = ctx.enter_context(tc.tile_pool(name="psB", bufs=2, space="PSUM"))

        for j in range(EXP):
            t0 = j * S
            hbig = hpool.tile([P, KO2 * S], BF16, tag="ht")
            for fo in range(FO1):
                w1t = w1pool.tile([P, KO1, P], BF16, tag="w1t")
                nc.sync.dma_start(w1t[:], w1s[j, fo])
                pa = psA.tile([P, S], F32, tag="psA")
                for ko in range(KO1):
                    for a, b in _chunks(S):
                        nc.tensor.matmul(
                            pa[:, a:b],
                            lhsT=w1t[:, ko, :],
                            rhs=xTe_s[:, ko * Tc + t0 + a : ko * Tc + t0 + b],
                            start=(ko == 0),
                            stop=(ko == KO1 - 1),
                        )
                nc.scalar.activation(
                    hbig[:, fo * S : (fo + 1) * S], pa[:], GELU, bias=b1s_s[:, j, fo : fo + 1]
                )
            for fo2 in range(FO2):
                w2t = w2pool.tile([P, KO2, P], BF16, tag="w2t")
                nc.sync.dma_start(w2t[:], w2s[j, fo2])
                pb = psB.tile([P, S], F32, tag="psB")
                for ko in range(KO2):
                    for a, b in _chunks(S):
                        nc.tensor.matmul(
                            pb[:, a:b],
                            lhsT=w2t[:, ko, :],
                            rhs=hbig[:, ko * S + a : ko * S + b],
                            start=(ko == 0),
                            stop=(ko == KO2 - 1),
                        )
                st = spool.tile([P, S], F32, tag="st")
                nc.scalar.activation(
                    st[:], pb[:], AF.Tanh, bias=b2s_s[:, j, fo2 : fo2 + 1], scale=0.5
                )
                # out = w*sigmoid(o) = wb2 + wb2*tanh
                tmp = tpool.tile([P, S], F32, tag="tmp")
                nc.vector.tensor_tensor(tmp[:], st[:], wb2[:, t0 : t0 + S], OP.mult)
                nc.vector.tensor_tensor(tmp[:], tmp[:], wb2[:, t0 : t0 + S], OP.add)
                nc.sync.dma_start(oT[:, fo2, t0 : t0 + S], tmp[:])
    nc.compile()
    return nc


def route(logits):
    """Host softmax/top-2/normalize + load-proportional slot assignment.

    32 uniform slots of capacity S; expert e gets k_e slots chosen greedily
    to minimize max per-slot load, so heavy experts spread over more slots."""
    lg = logits.astype(np.float32)
    m = lg.max(axis=1, keepdims=True)
    p = np.exp(lg - m)
    p /= p.sum(axis=1, keepdims=True)
    order = np.argsort(-p, axis=1, kind="stable")
    i1, i2 = order[:, 0], order[:, 1]
    r = np.arange(lg.shape[0])
    w1 = p[r, i1]
    w2 = p[r, i2]
    s = w1 + w2
    w1, w2 = w1 / s, w2 / s

    toks, wts = [], []
    for e in range(NE):
        t1 = np.nonzero(i1 == e)[0]
        t2 = np.nonzero(i2 == e)[0]
        toks.append(np.concatenate([t1, t2]))
        wts.append(np.concatenate([w1[t1], w2[t2]]).astype(np.float32))
    cnt = np.array([len(t) for t in toks])

    SLOTS = NCORES * EXP
    k = np.ones(NE, np.int64)
    for _ in range(SLOTS - NE):
        j = np.argmax(-(-cnt // k))  # expert with largest per-slot load
        k[j] += 1
    S = max(int(((max(-(-cnt // k)) + 31) // 32) * 32), 32)

    slot_t = np.zeros((SLOTS, S), np.int64)
    slot_w = np.zeros((SLOTS, S), np.float32)
    slot_e = np.zeros(SLOTS, np.int64)
    g = 0
    for e in range(NE):
        parts = np.array_split(np.arange(cnt[e]), k[e])
        for pt in parts:
            slot_e[g] = e
            slot_t[g, : len(pt)] = toks[e][pt]
            slot_w[g, : len(pt)] = wts[e][pt]
            g += 1
    assert g == SLOTS
    return slot_t, slot_w, slot_e, S


def kernel_sparse(**inputs):
    bf16 = mybir.dt.np(BF16)
    shared, xTs, _ = prep_inputs(inputs)

    if "gate" not in _NC_CACHE:
        _NC_CACHE["gate"] = build_nc_gate()
    ncg = _NC_CACHE["gate"]
    gmaps = [
        {
            "xT": xTs[c],
            "gw1": shared["gw1"],
            "gb1": shared["gb1"],
            "gw2": shared["gw2"],
            "gb2r": shared["gb2r"],
        }
        for c in range(NCORES)
    ]
    gres = run_bass_kernel_spmd(ncg, gmaps, core_ids=list(range(NCORES)))
    logits = np.concatenate(
        [gres.results[c]["lg"].transpose(1, 0, 2).reshape(T, NE) for c in range(NCORES)]
    )

    slot_t, slot_w, slot_e, S = route(logits)
    if S > S_MAX:  # extremely unbalanced routing: use the dense path
        return kernel_dense(**inputs)
    Tc = EXP * S

    if ("exp", S) not in _NC_CACHE:
        _NC_CACHE[("exp", S)] = build_nc_exp(S)
    nce = _NC_CACHE[("exp", S)]

    combined = np.asarray(inputs["combined"], np.float32)
    emaps = []
    for c in range(NCORES):
        gids = [EXP * c + j for j in range(EXP)]
        eids = [int(slot_e[g]) for g in gids]
        toks = np.concatenate([slot_t[g] for g in gids])
        ws = np.concatenate([slot_w[g] for g in gids])
        xg = combined[toks]
        emaps.append(
            {
                "xTe": np.ascontiguousarray(
                    xg.T.reshape(KO1, P, Tc).transpose(1, 0, 2)
                ).astype(bf16),
                "wrow": ws.reshape(1, Tc).astype(np.float32),
                "w1s": np.ascontiguousarray(shared["w1e"][eids]),
                "b1s": np.ascontiguousarray(shared["b1e"][:, eids, :]),
                "w2s": np.ascontiguousarray(shared["w2e"][eids]),
                "b2s": np.ascontiguousarray(shared["b2e"][:, eids, :]),
            }
        )
    _NC_CACHE["last_emaps"] = emaps
    eres = run_bass_kernel_spmd(nce, emaps, core_ids=list(range(NCORES)))

    fused = np.zeros((N, E), np.float32)
    for c in range(NCORES):
        rows = eres.results[c]["oT"].transpose(2, 1, 0).reshape(Tc, E)
        for j in range(EXP):
            g = EXP * c + j
            # np.add.at: padding reuses token 0 with an all-zero row
            np.add.at(fused, slot_t[g], rows[j * S : (j + 1) * S])
    return fused


MODE = "sparse"


def kernel(**inputs):
    if MODE == "sparse":
        try:
            return kernel_sparse(**inputs)
        except Exception:
            return kernel_dense(**inputs)
    return kernel_dense(**inputs)


if __name__ == "__main__":  # dev smoke test only; harness imports kernel()
    import reference  # noqa: PLC0415 -- not needed when imported as a module

    inputs = {k: np.asarray(v) for k, v in reference.setup_inputs().items()}
    out = kernel(**inputs)
    print(out.shape, out.dtype)

